# revision 13
# baseline (speedup 1.0000x reference)
"""Trainium2 Bass kernel for nn_CrossPairMemory.

Sharding: data-parallel over batch across 8 NeuronCores (512 rows each),
weights replicated per core, no collectives.

Algebraic restructuring (all folds are weight-only, done host-side in fp32):
  * The fusion first Linear collapses through the associative memory read:
      h = [A_P | A_M] @ C,  C = [[vP @ W1_top + b1], [vM @ W1_bot]]
    where A_* are the (Bc, 64) attention matrices.  This removes the
    26 GFLOP/core (Bc,7168)x(7168,3584) matmul entirely.
  * LayerNorm-1 statistics come from the same algebra:
      sum_f h = c1^T a      with c1 = C.sum(axis=1)
      sum_f h^2 = a^T G a   with G = C @ C^T   (kept in fp32 on device)
    so h is never materialized pre-norm.
  * LayerNorm-1 apply is folded into the mm1 matmul: the attention matrix
    is scaled per-column by rstd, C is pre-scaled per-feature by ln_g, and
    the -mu*rstd*ln_g offset enters via a K=1 rank-1 matmul into the same
    PSUM accumulation; gelu(scale+bias) reads PSUM directly.
  * The second fusion Linear and the per-pair output Linear collapse:
      W2' = W2 @ blockdiag(pair_w[:,128:,:]),  b' = b2 @ blockdiag(..) + pair_b
    so one (Bc,3584)x(3584,3584) matmul plus a small pair_states @ pw_top
    term produces the pre-LN per-pair outputs directly, batch-major.

Input-adaptive fast paths (checked on the actual arrays, general fallback):
skip the final LN scale/shift when pair_ln_g==1 and pair_ln_b==0, and skip
the stage-C bias matmul when the folded bias is exactly zero.
"""

import sys

for _p in ("/opt/trn_rl_repo",):
    if _p not in sys.path:
        sys.path.insert(0, _p)

import numpy as np
import ml_dtypes

import concourse.bass as bass
import concourse.tile as tile
from concourse import bacc, mybir
from concourse import bass_utils

BF = ml_dtypes.bfloat16
dt = mybir.dt
AF = mybir.ActivationFunctionType
ALU = mybir.AluOpType

NCORES = 8
B, P, PD, MD, S = 4096, 28, 128, 256, 64
D = P * PD            # 3584
Bc = B // NCORES      # 512 batch rows per core
NBT = Bc // PD        # 4 batch tiles of 128
MG = 7                # mm2 column groups of 4 pairs (512 cols)
EPS = 1e-5


def _build(unit_ln2, zero_bias):
    nc = bacc.Bacc(
        "TRN2", target_bir_lowering=False, debug=False, num_devices=NCORES
    )

    def din(name, shape, dty):
        return nc.dram_tensor(name, list(shape), dty, kind="ExternalInput").ap()

    psT = din("psT", (P, PD, Bc), dt.bfloat16)      # pair_states^T per pair
    msT = din("msT", (2, PD, Bc), dt.bfloat16)      # macro_state^T, 2 tiles
    kP = din("kP", (PD, S), dt.bfloat16)            # pair keys^T, pre-scaled
    kM = din("kM", (2, PD, S), dt.bfloat16)         # macro keys^T, pre-scaled
    Cg = din("Cg", (PD, D), dt.bfloat16)            # C * ln1_g, slot-major
    c1 = din("c1", (PD, 1), dt.float32)             # C row-sums
    Gm = din("Gm", (PD, PD), dt.float32)            # C @ C^T
    grow = din("grow", (1, D), dt.bfloat16)         # ln1_g row
    be1t = din("be1t", (PD, P), dt.float32)         # ln1_b, feature-major
    w2p = din("w2p", (MG, PD, P, 4 * PD), dt.bfloat16)  # W2' blocks
    pwt = din("pwt", (PD, P, PD), dt.bfloat16)      # pair_w top half, d-major
    if not zero_bias:
        bprow = din("bprow", (1, D), dt.bfloat16)   # b2 @ pw_bot + pair_b
    if not unit_ln2:
        g2bc = din("g2bc", (PD, P, PD), dt.float32)  # pair_ln_g broadcast
        b2bc = din("b2bc", (PD, P, PD), dt.float32)  # pair_ln_b broadcast
    out = nc.dram_tensor(
        "out", [Bc, D], dt.float32, kind="ExternalOutput"
    ).ap()

    with tile.TileContext(nc) as tc:
        with (
            tc.tile_pool(name="const", bufs=1) as const,
            tc.tile_pool(name="res", bufs=1) as res,
            tc.tile_pool(name="gres", bufs=1) as gres,
            tc.tile_pool(name="w2s", bufs=2) as pw2,
        ):
            ones_col_b = const.tile([PD, 1], dt.bfloat16, tag="ocb", name="ocb")
            nc.vector.memset(ones_col_b, 1.0)
            ones_col_f = const.tile([PD, 1], dt.float32, tag="ocf", name="ocf")
            nc.vector.memset(ones_col_f, 1.0)
            ones_row_b = const.tile([1, PD], dt.bfloat16, tag="orb", name="orb")
            nc.vector.memset(ones_row_b, 1.0)
            ones_row_f = const.tile([1, PD], dt.float32, tag="orf", name="orf")
            nc.vector.memset(ones_row_f, 1.0)
            eps_t = const.tile([PD, 1], dt.float32, tag="eps", name="eps")
            nc.vector.memset(eps_t, EPS)
            warm = const.tile([PD, Bc], dt.bfloat16, tag="warm", name="warm")
            nc.vector.memset(warm, 0.0)

            cst = {}

            def cload(nm, src, shp, dty):
                t = const.tile(list(shp), dty, tag=nm, name=nm)
                nc.sync.dma_start(t, src)
                cst[nm] = t

            # DMAs in consumption order: scores path first, stage C last.
            cload("kP", kP, (PD, S), dt.bfloat16)
            kM_sb, ms_sb = [], []
            for i in range(2):
                t = const.tile([PD, S], dt.bfloat16, tag=f"kM{i}", name=f"kM{i}")
                nc.sync.dma_start(t, kM[i])
                kM_sb.append(t)
                t = const.tile([PD, Bc], dt.bfloat16, tag=f"ms{i}", name=f"ms{i}")
                nc.sync.dma_start(t, msT[i])
                ms_sb.append(t)
            psT_sb = []
            for p in range(P):
                t = res.tile([PD, Bc], dt.bfloat16, tag=f"psT{p}", name=f"psT{p}")
                nc.sync.dma_start(t, psT[p])
                psT_sb.append(t)
            cload("Cg", Cg, (PD, D), dt.bfloat16)
            cload("c1", c1, (PD, 1), dt.float32)
            cload("Gm", Gm, (PD, PD), dt.float32)
            cload("grow", grow, (1, D), dt.bfloat16)
            cload("be1t", be1t, (PD, P), dt.float32)
            # prefetch first two W2' blocks behind the front-critical loads
            w2tiles = {}
            for mg in range(2):
                t = pw2.tile([PD, P, 4 * PD], dt.bfloat16, tag="w2b", name="w2b")
                nc.sync.dma_start(t, w2p[mg])
                w2tiles[mg] = t
            cload("pwt", pwt, (PD, P, PD), dt.bfloat16)
            if not zero_bias:
                cload("bprow", bprow, (1, D), dt.bfloat16)
            if not unit_ln2:
                cload("g2bc", g2bc, (PD, P, PD), dt.float32)
                cload("b2bc", b2bc, (PD, P, PD), dt.float32)

            # post-gelu activations, feature-major k-tiles (mm2 stationary)
            gsb = [
                gres.tile([PD, Bc], dt.bfloat16, tag=f"g{n}", name=f"g{n}")
                for n in range(P)
            ]

            # ------- front: memory read + LN1 + gelu, 2 batch chunks -----
            # The two 256-col chunks are interleaved phase-by-phase so one
            # chunk's serial row-op latency chain hides under the other
            # chunk's matmuls (engine queues execute in program order).
            CH, CSZ = 2, Bc // 2
            chs = [slice(c * CSZ, (c + 1) * CSZ) for c in range(CH)]
            with (
                tc.tile_pool(name="fr", bufs=1) as fr,
                tc.tile_pool(name="psSp", bufs=2, space="PSUM") as ppsp,
                tc.tile_pool(name="psRw", bufs=2, space="PSUM") as pprw,
                tc.tile_pool(name="psSt", bufs=2, space="PSUM") as ppst,
                tc.tile_pool(name="psM1", bufs=2, space="PSUM") as ppm1,
            ):
                def ftile(nm, ch, shape, dty):
                    return fr.tile(list(shape), dty, tag=f"{nm}{ch}",
                                   name=f"{nm}{ch}")

                # spin the PE p-state up while input DMAs stream
                for _ in range(4):
                    wps = ppm1.tile([PD, CSZ], dt.float32, tag="pm", name="wps")
                    nc.tensor.matmul(wps, warm[:, 0:PD], warm[:, 0:CSZ],
                                     start=True, stop=True)

                eb, ab, abf, apr, nmr = {}, {}, {}, {}, {}
                den_t, rr_t, rbc_t, st_t = {}, {}, {}, {}
                # phase A: score matmuls + exp
                for ch, cs in enumerate(chs):
                    spM = ppsp.tile([S, CSZ], dt.float32, tag="sp", name="spM")
                    nc.tensor.matmul(spM, kM_sb[0], ms_sb[0][:, cs],
                                     start=True, stop=False)
                    nc.tensor.matmul(spM, kM_sb[1], ms_sb[1][:, cs],
                                     start=False, stop=True)
                    eb[ch, "M"] = ftile("ebM", ch, (S, CSZ), dt.bfloat16)
                    nc.scalar.activation(eb[ch, "M"], spM, AF.Exp)
                    spP = ppsp.tile([S, CSZ], dt.float32, tag="sp", name="spP")
                    for p in range(P):
                        nc.tensor.matmul(spP, cst["kP"], psT_sb[p][:, cs],
                                         start=(p == 0), stop=(p == P - 1))
                    eb[ch, "P"] = ftile("ebP", ch, (S, CSZ), dt.bfloat16)
                    nc.scalar.activation(eb[ch, "P"], spP, AF.Exp)
                # phase B: softmax normalization, chunk-interleaved
                for ch in range(CH):
                    for w in ("M", "P"):
                        den_t[ch, w] = pprw.tile([1, CSZ], dt.float32,
                                                 tag="row", name=f"den{w}{ch}")
                        nc.tensor.matmul(den_t[ch, w], ones_col_b[0:S, :],
                                         eb[ch, w], start=True, stop=True)
                for ch in range(CH):
                    for w in ("M", "P"):
                        rr_t[ch, w] = ftile(f"rr{w}", ch, (1, CSZ), dt.float32)
                        nc.vector.reciprocal(rr_t[ch, w], den_t[ch, w])
                for ch in range(CH):
                    for w in ("M", "P"):
                        rbc_t[ch, w] = ppsp.tile([S, CSZ], dt.float32,
                                                 tag="sp", name=f"rbc{w}{ch}")
                        nc.tensor.matmul(rbc_t[ch, w], ones_row_f[:, 0:S],
                                         rr_t[ch, w], start=True, stop=True)
                for ch in range(CH):
                    ab[ch] = ftile("abPM", ch, (PD, CSZ), dt.bfloat16)
                    nc.vector.tensor_mul(ab[ch][S:2 * S, :], eb[ch, "M"],
                                         rbc_t[ch, "M"])
                    nc.vector.tensor_mul(ab[ch][0:S, :], eb[ch, "P"],
                                         rbc_t[ch, "P"])
                    abf[ch] = ftile("abF", ch, (PD, CSZ), dt.float32)
                    nc.scalar.activation(abf[ch], ab[ch], AF.Copy)
                # phase C: LN1 statistics from the attention algebra
                for ch in range(CH):
                    st_t[ch, "mu"] = pprw.tile([1, CSZ], dt.float32,
                                               tag="row", name=f"mu{ch}")
                    nc.tensor.matmul(st_t[ch, "mu"], cst["c1"], abf[ch],
                                     start=True, stop=True)
                    st_t[ch, "Gt"] = ppst.tile([PD, CSZ], dt.float32,
                                               tag="big", name=f"Gt{ch}")
                    nc.tensor.matmul(st_t[ch, "Gt"], cst["Gm"], abf[ch],
                                     start=True, stop=True)
                for ch in range(CH):
                    qq = ftile("qq", ch, (PD, CSZ), dt.float32)
                    nc.vector.tensor_mul(qq, abf[ch], st_t[ch, "Gt"])
                    st_t[ch, "sq"] = pprw.tile([1, CSZ], dt.float32,
                                               tag="row", name=f"sq{ch}")
                    nc.tensor.matmul(st_t[ch, "sq"], ones_col_f, qq,
                                     start=True, stop=True)
                for ch in range(CH):
                    m = ftile("m", ch, (1, CSZ), dt.float32)
                    nc.vector.tensor_scalar_mul(m, st_t[ch, "mu"], 1.0 / D)
                    m2 = ftile("m2", ch, (1, CSZ), dt.float32)
                    nc.vector.tensor_mul(m2, m, m)
                    var = ftile("var", ch, (1, CSZ), dt.float32)
                    nc.vector.scalar_tensor_tensor(
                        var, st_t[ch, "sq"], 1.0 / D, m2,
                        op0=ALU.mult, op1=ALU.subtract)
                    sd = ftile("sd", ch, (1, CSZ), dt.float32)
                    nc.scalar.activation(sd, var, AF.Sqrt,
                                         bias=eps_t[0:1, :], scale=1.0)
                    rstd = ftile("rstd", ch, (1, CSZ), dt.float32)
                    nc.vector.reciprocal(rstd, sd)
                    nmr[ch] = ftile("negmr", ch, (1, CSZ), dt.bfloat16)
                    nc.vector.scalar_tensor_tensor(
                        nmr[ch], m, -1.0, rstd, op0=ALU.mult, op1=ALU.mult)
                    rstd_bc = ppst.tile([PD, CSZ], dt.float32, tag="big",
                                        name=f"rbc2{ch}")
                    nc.tensor.matmul(rstd_bc, ones_row_f, rstd,
                                     start=True, stop=True)
                    apr[ch] = ftile("apr", ch, (PD, CSZ), dt.bfloat16)
                    nc.vector.tensor_mul(apr[ch], ab[ch], rstd_bc)
                # phase D: mm1' with LN1 apply folded in, gelu from PSUM
                for ch, cs in enumerate(chs):
                    for n in range(P):
                        nsl = slice(n * PD, (n + 1) * PD)
                        pm = ppm1.tile([PD, CSZ], dt.float32, tag="pm",
                                       name="pm")
                        nc.tensor.matmul(pm, cst["grow"][:, nsl], nmr[ch],
                                         start=True, stop=False)
                        nc.tensor.matmul(pm, cst["Cg"][:, nsl], apr[ch],
                                         start=False, stop=True)
                        nc.scalar.activation(
                            gsb[n][:, cs], pm, AF.Gelu,
                            bias=cst["be1t"][:, n:n + 1], scale=1.0,
                        )

            # ------------- stage BC: mm2' + pair_states part + LN2 ------
            with (
                tc.tile_pool(name="yo", bufs=3) as pyo,
                tc.tile_pool(name="sc", bufs=4) as psc,
                tc.tile_pool(name="psC", bufs=3, space="PSUM") as ppc,
            ):
                for mg in range(MG):
                    w2b = w2tiles.pop(mg)
                    mgsl = slice(mg * 4 * PD, (mg + 1) * 4 * PD)
                    for bt in range(NBT):
                        bs = slice(bt * PD, (bt + 1) * PD)
                        po = ppc.tile([PD, 4 * PD], dt.float32, tag="po",
                                      name="po")
                        # the accumulation leader must write the FULL bank
                        # width with start=True: hardware start zeroes the
                        # whole 2KB PSUM zero-region, not just written cols
                        if zero_bias:
                            nc.tensor.matmul(
                                po, gsb[0][:, bs], w2b[:, 0, :],
                                start=True, stop=False,
                            )
                        else:
                            nc.tensor.matmul(
                                po, ones_row_b, cst["bprow"][:, mgsl],
                                start=True, stop=False,
                            )
                        for s in range(4):
                            pidx = 4 * mg + s
                            nc.tensor.matmul(
                                po[:, s * PD:(s + 1) * PD],
                                psT_sb[pidx][:, bs], cst["pwt"][:, pidx, :],
                                start=False, stop=False,
                                skip_group_check=True,
                            )
                        for k in range(1 if zero_bias else 0, P):
                            nc.tensor.matmul(
                                po, gsb[k][:, bs], w2b[:, k, :],
                                start=False, stop=(k == P - 1),
                                skip_group_check=True,
                            )
                        y4 = pyo.tile([PD, 4 * PD], dt.float32, tag="y4",
                                      name="y4")
                        mva = psc.tile([PD, 8], dt.float32, tag="mv", name="mv")
                        rst4 = psc.tile([PD, 4], dt.float32, tag="rst4",
                                        name="rst4")
                        for s in range(4):
                            st6 = psc.tile([PD, 6], dt.float32, tag="st6",
                                           name="st6")
                            nc.vector.bn_stats(st6, po[:, s * PD:(s + 1) * PD])
                            nc.vector.bn_aggr(
                                mva[:, 2 * s:2 * s + 2], st6)
                            sd2 = psc.tile([PD, 1], dt.float32, tag="sd2",
                                           name="sd2")
                            nc.scalar.activation(
                                sd2, mva[:, 2 * s + 1:2 * s + 2], AF.Sqrt,
                                bias=eps_t, scale=1.0)
                            nc.vector.reciprocal(rst4[:, s:s + 1], sd2)
                        for s in range(4):
                            pidx = 4 * mg + s
                            ssl = slice(s * PD, (s + 1) * PD)
                            if unit_ln2:
                                nc.vector.tensor_scalar(
                                    y4[:, ssl], po[:, ssl],
                                    mva[:, 2 * s:2 * s + 1],
                                    rst4[:, s:s + 1],
                                    op0=ALU.subtract, op1=ALU.mult,
                                )
                            else:
                                tn = psc.tile([PD, PD], dt.float32, tag="tn",
                                              name="tn")
                                nc.vector.tensor_scalar(
                                    tn, po[:, ssl],
                                    mva[:, 2 * s:2 * s + 1],
                                    rst4[:, s:s + 1],
                                    op0=ALU.subtract, op1=ALU.mult,
                                )
                                tg = psc.tile([PD, PD], dt.float32, tag="tg",
                                              name="tg")
                                nc.vector.tensor_mul(
                                    tg, tn, cst["g2bc"][:, pidx, :])
                                nc.vector.tensor_add(
                                    y4[:, ssl], tg, cst["b2bc"][:, pidx, :])
                        nc.sync.dma_start(out[bs, mgsl], y4)
                    # prefetch two blocks ahead; emitted after this block's
                    # reads so the rotated buffer is overwrite-safe
                    if mg + 2 < MG:
                        t = pw2.tile([PD, P, 4 * PD], dt.bfloat16, tag="w2b",
                                     name="w2b")
                        nc.sync.dma_start(t, w2p[mg + 2])
                        w2tiles[mg + 2] = t

    nc.compile()
    return nc


_CACHE = {}


def _get_nc(unit_ln2, zero_bias):
    key = (unit_ln2, zero_bias)
    if key not in _CACHE:
        _CACHE[key] = _build(unit_ln2, zero_bias)
    return _CACHE[key]


def _prep_in_maps(inputs):
    f32 = np.float32
    g = lambda k: np.asarray(inputs[k], f32)

    psT_full = np.asarray(g("pair_states").transpose(1, 2, 0), dtype=BF)  # [P,PD,B]
    msT_full = np.asarray(g("macro_state").T, dtype=BF)                   # [MD,B]

    W1 = g("fusion_w1")                       # (7168, 3584)
    C = np.concatenate(
        [
            g("mem_pair_vals") @ W1[:D] + g("fusion_b1")[None, :],
            g("mem_macro_vals") @ W1[D:],
        ],
        axis=0,
    )                                          # (128, 3584)
    g1 = g("fusion_ln_g")
    pw = g("pair_w")                           # (28, 256, 128)
    pwA, pwB = pw[:, :PD, :], pw[:, PD:, :]
    # W2' = W2 @ blockdiag(pwB): (3584, 28, 128)
    W2r = g("fusion_w2").reshape(D, P, PD)
    W2p = np.matmul(W2r.transpose(1, 0, 2), pwB)          # (28, 3584, 128)
    W2p = W2p.transpose(1, 0, 2).reshape(D, D)
    bp = (
        np.einsum("pc,pce->pe", g("fusion_b2").reshape(P, PD), pwB)
        + g("pair_b")
    ).reshape(1, D)

    import os
    ln2g, ln2b = g("pair_ln_g"), g("pair_ln_b")
    unit_ln2 = bool((ln2g == 1.0).all() and (ln2b == 0.0).all())
    zero_bias = bool((bp == 0.0).all())
    if os.environ.get("K_NOFAST"):
        unit_ln2 = zero_bias = False

    shared = {
        "kP": np.ascontiguousarray(
            (g("mem_pair_keys").T / (P * np.sqrt(PD))).astype(BF)),
        "kM": np.ascontiguousarray(
            (g("mem_macro_keys").T / np.sqrt(MD)).reshape(2, PD, S).astype(BF)),
        "Cg": np.ascontiguousarray((C * g1[None, :]).astype(BF)),
        "c1": np.ascontiguousarray(C.sum(axis=1, dtype=np.float64)
                                   .astype(f32).reshape(PD, 1)),
        "Gm": np.ascontiguousarray((C @ C.T).astype(f32)),
        "grow": np.ascontiguousarray(g1.reshape(1, D).astype(BF)),
        "be1t": np.ascontiguousarray(g("fusion_ln_b").reshape(P, PD).T),
        "w2p": np.ascontiguousarray(
            W2p.reshape(P, PD, MG, 4 * PD).transpose(2, 1, 0, 3).astype(BF)),
        "pwt": np.ascontiguousarray(pwA.transpose(1, 0, 2).astype(BF)),
    }
    if not zero_bias:
        shared["bprow"] = np.ascontiguousarray(bp.astype(BF))
    if not unit_ln2:
        shared["g2bc"] = np.ascontiguousarray(
            np.broadcast_to(ln2g[None], (PD, P, PD)))
        shared["b2bc"] = np.ascontiguousarray(
            np.broadcast_to(ln2b[None], (PD, P, PD)))
    in_maps = []
    for c in range(NCORES):
        m = dict(shared)
        m["psT"] = np.ascontiguousarray(psT_full[:, :, c * Bc:(c + 1) * Bc])
        m["msT"] = np.ascontiguousarray(
            msT_full[:, c * Bc:(c + 1) * Bc].reshape(2, PD, Bc))
        in_maps.append(m)
    return in_maps, unit_ln2, zero_bias


def _run(inputs, trace=False):
    in_maps, unit_ln2, zero_bias = _prep_in_maps(inputs)
    nc = _get_nc(unit_ln2, zero_bias)
    res = bass_utils.run_bass_kernel_spmd(
        nc, in_maps, core_ids=list(range(NCORES)), trace=trace
    )
    outp = np.concatenate(
        [res.results[c]["out"] for c in range(NCORES)], axis=0
    ).reshape(B, P, PD)
    return np.ascontiguousarray(outp.astype(np.float32)), res


def kernel(**inputs):
    outp, _ = _run(inputs, trace=False)
    return outp
